# revision 13
# baseline (speedup 1.0000x reference)
"""Causal self-attention on 8 trn2 NeuronCores.

Sharding (batch+head hint): core c handles batch b = c//2 (data parallel)
and head-group g = c%2 (8 of 16 heads; tensor-parallel slice of w_qkv
columns / w_out rows). Each core computes a full-batch-slice partial of the
output projection over its 512 head dims; the two partials per batch are
summed on gather (the "all-reduce after out_proj").

Kernel dataflow per core (S=2048 tokens, D=1024, 8 heads x 64):
  phase 1: x^T comes pre-transposed from the host. qT/kT = W^T @ x^T
           (stationary weights, moving tokens) in [64h, S] layouts; v is
           computed in NATURAL [tokens, vdim] orientation (stationary xT
           token chunks, moving Wv) and copied with a 66-col stride into
           v1 so each (head, chunk) block carries an appended ones column
           (denominator trick; v1 pre-memset to 1.0).
  phase 2: per head PAIR (even head on PE rows 0:64, odd head on rows
           64:128, adjacent matmuls -> concurrent row-group execution),
           exact-causal flash attention in transposed orientation:
           scoresT[k,q] pair -> one 2-bank PSUM tile; ONE ScalarE exp per
           pair (no max subtraction; scores O(N(0,1)) are fp32-safe);
           causal mask applied post-exp as a bf16 multiply on the diagonal
           chunks; out_hT[dh,q] += v1_blk.T @ probsT with the stationary
           padded to 128 cols (FWL-eligible weight loads), ones row at
           partition 64 accumulates the denominator; normalize via
           reciprocal + gpsimd partition_broadcast.
  phase 3: partial out = oT.T @ Wout (per 128-token chunk), PSUM->SBUF on
           ScalarE, paired 4KB-row DMA stores.
"""
import numpy as np

B = 4
S = 2048
D = 1024
HG = 8           # heads per core
DH = 64
NCORES = 8
NB = S // 512    # 512-token q blocks
KC = D // 128    # contraction chunks over D
VBLK = 72        # v1 block stride: 64 v cols + ones col + pad (16B-aligned)

_CACHE = {}


def _build_nc():
    import concourse.bass as bass  # noqa
    import concourse.mybir as mybir
    import concourse.tile as tile
    from concourse import bacc

    F32 = mybir.dt.float32
    BF = mybir.dt.bfloat16
    Exp = mybir.ActivationFunctionType.Exp

    nc = bacc.Bacc("TRN2", target_bir_lowering=False, debug=False,
                   enable_asserts=False, num_devices=NCORES)
    xT_d = nc.dram_tensor("xT", [D, S], BF, kind="ExternalInput")
    wqkv_d = nc.dram_tensor("wqkv", [D, 3 * 512], BF, kind="ExternalInput")
    wout_d = nc.dram_tensor("wout", [512, D], BF, kind="ExternalInput")
    masks_d = nc.dram_tensor("masks", [128, 256], BF, kind="ExternalInput")
    out_d = nc.dram_tensor("out", [S, D], F32, kind="ExternalOutput")

    with tile.TileContext(nc) as tc:
        with tc.tile_pool(name="persist", bufs=1) as persist, \
             tc.tile_pool(name="xT", bufs=2) as xT_pool, \
             tc.tile_pool(name="probs", bufs=6) as pr_pool, \
             tc.tile_pool(name="recip", bufs=2) as rc_pool, \
             tc.tile_pool(name="rbc", bufs=2) as rb_pool, \
             tc.tile_pool(name="obig", bufs=1) as obig_pool, \
             tc.tile_pool(name="ostage", bufs=3) as ost_pool, \
             tc.tile_pool(name="ps_sc", bufs=1, space="PSUM") as ps_sc, \
             tc.tile_pool(name="ps_wk", bufs=2, space="PSUM") as ps_wk, \
             tc.tile_pool(name="ps_out", bufs=2, space="PSUM") as ps_out:
            qT = persist.tile([128, 4 * S], BF)
            kT = persist.tile([128, 4 * S], BF)
            # v1: per (head h, k-chunk sck) a [128, 66] block at col
            # (h*16+sck)*66: cols 0:64 = v values, col 64 = ones (denom),
            # col 65 pad; +128 tail pad for the 128-col padded stationary.
            v1 = persist.tile([128, HG * 16 * VBLK + 128], BF)
            oT = obig_pool.tile([128, 4 * S], BF)
            nc.vector.memset(v1[:], 1.0)
            tri2 = persist.tile([128, 256], BF)
            nc.sync.dma_start(tri2[:], masks_d.ap())

            xT_tiles = {}

            def dma_x(tb):
                t = xT_pool.tile([128, KC * 512], BF, tag="xT",
                                 name=f"xTd_{tb}")
                for ki in range(KC):
                    nc.sync.dma_start(
                        t[:, ki * 512:(ki + 1) * 512],
                        xT_d[ki * 128:(ki + 1) * 128,
                             tb * 512:(tb + 1) * 512])
                xT_tiles[tb] = t

            # interleave x(0) and w-qk chunk DMAs so the first matmul chain
            # trickles in with the DMA stream; defer the w-v half + wout
            w_sb = persist.tile([128, KC * 1536], BF)
            xt0 = xT_pool.tile([128, KC * 512], BF, tag="xT", name="xTd_0")
            xT_tiles[0] = xt0
            for ki in range(KC):
                nc.sync.dma_start(
                    xt0[:, ki * 512:(ki + 1) * 512],
                    xT_d[ki * 128:(ki + 1) * 128, 0:512])
                nc.sync.dma_start(
                    w_sb[:, ki * 1536: ki * 1536 + 1024],
                    wqkv_d[ki * 128:(ki + 1) * 128, 0:1024])
            for ki in range(KC):
                nc.sync.dma_start(
                    w_sb[:, ki * 1536 + 1024:(ki + 1) * 1536],
                    wqkv_d[ki * 128:(ki + 1) * 128, 1024:1536])
            wout_sb = persist.tile([128, 4 * D], BF)

            def qkv_block(tb):
                xT_sb = xT_tiles.pop(tb)
                if tb + 1 < NB:
                    dma_x(tb + 1)
                # q (m 0..3) and k (m 4..7): stationary w chunk, moving xT
                for m in range(8):
                    acc = ps_wk.tile([128, 512], F32, tag="wk",
                                     name=f"acc_{tb}_{m}")
                    for ki in range(KC):
                        nc.tensor.matmul(
                            acc[:],
                            w_sb[:, ki * 1536 + m * 128: ki * 1536 + (m + 1) * 128],
                            xT_sb[:, ki * 512:(ki + 1) * 512],
                            start=(ki == 0), stop=(ki == KC - 1))
                    dst = qT if m < 4 else kT
                    r = m if m < 4 else m - 4
                    nc.vector.tensor_copy(
                        dst[:, r * S + tb * 512: r * S + tb * 512 + 512],
                        acc[:])
                # v natural: stationary xT token chunk, moving Wv
                for t in range(4):
                    vacc = ps_wk.tile([128, 512], F32, tag="wk",
                                      name=f"vacc_{tb}_{t}")
                    for ki in range(KC):
                        nc.tensor.matmul(
                            vacc[:],
                            xT_sb[:, ki * 512 + t * 128: ki * 512 + (t + 1) * 128],
                            w_sb[:, ki * 1536 + 1024: ki * 1536 + 1536],
                            start=(ki == 0), stop=(ki == KC - 1))
                    sck = tb * 4 + t
                    # strided copy: head h's 64 cols -> v1 block (h*16+sck)
                    v1v = v1[:, 0:HG * 16 * VBLK].rearrange(
                        "p (h c u) -> p h c u", h=HG, c=16)
                    nc.vector.tensor_copy(
                        v1v[:, :, sck:sck + 1, 0:64],
                        vacc[:].rearrange("p (h o u) -> p h o u", h=HG, o=1))

            def attention_block(tb, fillers=()):
                ns = 4 * tb + 4   # k chunks for this q block
                nd = 4 * tb       # non-diagonal chunk count (even)
                tri2v = tri2[:].rearrange("p (h u) -> p h u", h=2)
                out_ps = {}
                pr_refs = {}
                arenas = {}

                def emit_sc(r, s):
                    lo = max(128 * s - 512 * tb, 0)
                    n = 512 - lo
                    arena = arenas[r]
                    base = (s % 2) * 1024
                    for half in range(2):
                        po = 64 * half
                        nc.tensor.matmul(
                            arena[:, base + half * 512: base + half * 512 + n],
                            kT[po:po + 64, r * S + s * 128: r * S + s * 128 + 128],
                            qT[po:po + 64,
                               r * S + 512 * tb + lo: r * S + 512 * (tb + 1)],
                            start=True, stop=True)
                    if s < nd and s % 2 == 0:
                        return  # exp deferred: paired with s+1
                    if s < nd:
                        pr2 = pr_pool.tile([128, 2048], BF, tag="probs2",
                                           bufs=3, name=f"pr2_{tb}_{r}_{s}")
                        nc.scalar.activation(
                            pr2[:].rearrange("p (a h u) -> p a h u", a=2, h=2),
                            arena[:].rearrange("p (a h u) -> p a h u", a=2, h=2),
                            Exp)
                        pr_refs[(r, s - 1)] = (pr2, 0)
                        pr_refs[(r, s)] = (pr2, 1024)
                        return
                    # diagonal chunk: single exp + causal mask multiply
                    pr = pr_pool.tile([128, 1024], BF, tag="probs",
                                      bufs=4, name=f"pr_{tb}_{r}_{s}")
                    nc.scalar.activation(
                        pr[:].rearrange("p (h u) -> p h u", h=2)[:, :, 0:n],
                        arena[:, base:base + 1024]
                        .rearrange("p (h u) -> p h u", h=2)[:, :, 0:n],
                        Exp)
                    prv = pr[:].rearrange("p (h u) -> p h u", h=2)
                    nc.vector.tensor_mul(
                        prv[:, :, 0:128], prv[:, :, 0:128], tri2v)
                    pr_refs[(r, s)] = (pr, 0)

                def emit_out(r, s):
                    lo = max(128 * s - 512 * tb, 0)
                    n = 512 - lo
                    prt, off = pr_refs.pop((r, s))
                    for half in range(2):
                        h = 2 * r + half
                        if s == 0:
                            out_ps[h] = ps_out.tile(
                                [128, 512], F32, tag="o", name=f"ops_{tb}_{h}")
                        blk = (h * 16 + s) * VBLK
                        nc.tensor.matmul(
                            out_ps[h][:, lo:512],
                            v1[:, blk: blk + 128],
                            prt[:, off + half * 512: off + half * 512 + n],
                            start=(s == 0), stop=(s == ns - 1))
                    if s == ns - 1:
                        for half in range(2):
                            h = 2 * r + half
                            po = 64 * half
                            op = out_ps.pop(h)
                            den = rc_pool.tile([1, 512], F32, tag="den")
                            nc.vector.tensor_copy(den[:], op[64:65, :])
                            rc = rc_pool.tile([1, 512], F32, tag="rc")
                            nc.vector.reciprocal_approx_fast(rc[:], den[:])
                            rb = rb_pool.tile([64, 512], F32, tag="rb")
                            nc.gpsimd.partition_broadcast(rb[:], rc[:])
                            nc.vector.tensor_mul(
                                oT[po:po + 64,
                                   r * S + 512 * tb: r * S + 512 * tb + 512],
                                op[0:64, :], rb[:])

                LA = 2
                slots = [(r, s) for r in range(4) for s in range(ns)]
                fillers = list(fillers)
                stride = max(1, len(slots) // len(fillers)) if fillers else 0
                fi = 0
                for i, (r, s) in enumerate(slots):
                    if s == 0:
                        arenas[r] = ps_sc.tile([128, 2048], F32, tag="sc",
                                               name=f"scA_{tb}_{r}")
                    if fillers and fi < len(fillers) and i % stride == stride // 2:
                        fillers[fi]()
                        fi += 1
                    emit_sc(r, s)
                    if i >= LA:
                        emit_out(*slots[i - LA])
                for rs in slots[-LA:]:
                    emit_out(*rs)
                while fi < len(fillers):
                    fillers[fi]()
                    fi += 1

            def proj_chunk(m):
                ost = ost_pool.tile([128, 1024], F32, tag="ost",
                                    name=f"ost_{m}")
                for half in range(2):
                    pso = ps_wk.tile([128, 512], F32, tag="wk",
                                     name=f"pso_{m}_{half}")
                    for k in range(4):
                        nc.tensor.matmul(
                            pso[:],
                            oT[:, k * S + m * 128: k * S + m * 128 + 128],
                            wout_sb[:, k * D + half * 512: k * D + half * 512 + 512],
                            start=(k == 0), stop=(k == 3))
                    nc.vector.tensor_copy(
                        ost[:, half * 512:(half + 1) * 512], pso[:])
                nc.sync.dma_start(
                    out_d[m * 128:(m + 1) * 128, :], ost[:])

            def proj_fillers(j):
                return [lambda m=m: proj_chunk(m) for m in range(4 * j, 4 * j + 4)]

            for tb in range(NB):
                qkv_block(tb)
                if tb == 0:
                    nc.sync.dma_start(
                        wout_sb[:].rearrange("p (k n) -> p k n", k=4),
                        wout_d.ap().rearrange("(k p) n -> p k n", p=128),
                    )
                attention_block(
                    tb, proj_fillers(tb - 1) if tb >= 1 else ())
            for f in proj_fillers(NB - 1):
                f()
    nc.compile()
    return nc


def _make_masks():
    # tri2[p, j*128+c] = 1.0 if c >= p else 0 (keep-mask for the two
    # halves of a diagonal-chunk probs pair)
    p = np.arange(128)[:, None]
    c = np.arange(128)[None, :]
    tri = (c >= p).astype(np.float32)
    return np.concatenate([tri, tri], axis=1)


def _make_in_maps(x, w_qkv, w_out):
    import ml_dtypes
    bf = ml_dtypes.bfloat16
    masks = _make_masks().astype(bf)
    scale = np.float32(DH ** -0.5)
    in_maps = []
    for c in range(NCORES):
        g = c % 2
        wq = w_qkv[:, g * 512:(g + 1) * 512] * scale
        wk = w_qkv[:, D + g * 512: D + (g + 1) * 512]
        wv = w_qkv[:, 2 * D + g * 512: 2 * D + (g + 1) * 512]
        in_maps.append({
            "xT": np.ascontiguousarray(x[c // 2].T).astype(bf),
            "wqkv": np.ascontiguousarray(np.concatenate([wq, wk, wv], axis=1)).astype(bf),
            "wout": np.ascontiguousarray(w_out[g * 512:(g + 1) * 512, :]).astype(bf),
            "masks": masks,
        })
    return in_maps


def kernel(x, w_qkv, w_out):
    from concourse.bass_utils import run_bass_kernel_spmd

    x = np.asarray(x, dtype=np.float32)
    w_qkv = np.asarray(w_qkv, dtype=np.float32)
    w_out = np.asarray(w_out, dtype=np.float32)
    assert x.shape == (B, S, D) and w_qkv.shape == (D, 3 * D) and w_out.shape == (D, D)

    if "nc" not in _CACHE:
        _CACHE["nc"] = _build_nc()
    nc = _CACHE["nc"]

    in_maps = _make_in_maps(x, w_qkv, w_out)
    res = run_bass_kernel_spmd(nc, in_maps, core_ids=list(range(NCORES)),
                               trace=False)
    out = np.empty((B, S, D), dtype=np.float32)
    for b in range(B):
        out[b] = res.results[2 * b]["out"] + res.results[2 * b + 1]["out"]
    return out


# revision 16
# speedup vs baseline: 1.1508x; 1.1508x over previous
"""Causal self-attention on 8 trn2 NeuronCores.

Sharding (batch+head hint): core c handles batch b = c//2 (data parallel)
and head-group g = c%2 (8 of 16 heads; tensor-parallel slice of w_qkv
columns / w_out rows). Each core computes a full-batch-slice partial of the
output projection over its 512 head dims; the two partials per batch are
summed on gather (the "all-reduce after out_proj").

Kernel dataflow per core (S=2048 tokens, D=1024, 8 heads x 64):
  phase 1: x^T comes pre-transposed from the host. qT/kT = W^T @ x^T
           (stationary weights, moving tokens) in [64h, S] layouts; v is
           computed in NATURAL [tokens, vdim] orientation (stationary xT
           token chunks, moving Wv) and copied with a 66-col stride into
           v1 so each (head, chunk) block carries an appended ones column
           (denominator trick; v1 pre-memset to 1.0).
  phase 2: per head PAIR (even head on PE rows 0:64, odd head on rows
           64:128, adjacent matmuls -> concurrent row-group execution),
           exact-causal flash attention in transposed orientation:
           scoresT[k,q] pair -> one 2-bank PSUM tile; ONE ScalarE exp per
           pair (no max subtraction; scores O(N(0,1)) are fp32-safe);
           causal mask applied post-exp as a bf16 multiply on the diagonal
           chunks; out_hT[dh,q] += v1_blk.T @ probsT with the stationary
           padded to 128 cols (FWL-eligible weight loads), ones row at
           partition 64 accumulates the denominator; normalize via
           reciprocal + gpsimd partition_broadcast.
  phase 3: partial out = oT.T @ Wout (per 128-token chunk), PSUM->SBUF on
           ScalarE, paired 4KB-row DMA stores.
"""
import numpy as np

B = 4
S = 2048
D = 1024
HG = 8           # heads per core
DH = 64
NCORES = 8
NB = S // 512    # 512-token q blocks
KC = D // 128    # contraction chunks over D
VBLK = 72        # v1 block stride: 64 v cols + ones col + pad (16B-aligned)

_CACHE = {}


def _build_nc():
    import concourse.bass as bass  # noqa
    import concourse.mybir as mybir
    import concourse.tile as tile
    from concourse import bacc

    F32 = mybir.dt.float32
    BF = mybir.dt.bfloat16
    Exp = mybir.ActivationFunctionType.Exp

    nc = bacc.Bacc("TRN2", target_bir_lowering=False, debug=False,
                   enable_asserts=False, num_devices=NCORES)
    xT_d = nc.dram_tensor("xT", [D, S], BF, kind="ExternalInput")
    wqkv_d = nc.dram_tensor("wqkv", [D, 3 * 512], BF, kind="ExternalInput")
    wout_d = nc.dram_tensor("wout", [512, D], BF, kind="ExternalInput")
    masks_d = nc.dram_tensor("masks", [128, 256], BF, kind="ExternalInput")
    out_d = nc.dram_tensor("out", [S, D], F32, kind="ExternalOutput")

    with tile.TileContext(nc) as tc:
        with tc.tile_pool(name="persist", bufs=1) as persist, \
             tc.tile_pool(name="xT", bufs=2) as xT_pool, \
             tc.tile_pool(name="probs", bufs=6) as pr_pool, \
             tc.tile_pool(name="recip", bufs=2) as rc_pool, \
             tc.tile_pool(name="rbc", bufs=2) as rb_pool, \
             tc.tile_pool(name="obig", bufs=1) as obig_pool, \
             tc.tile_pool(name="ostage", bufs=3) as ost_pool, \
             tc.tile_pool(name="ps_sc", bufs=2, space="PSUM") as ps_sc, \
             tc.tile_pool(name="ps_wk", bufs=2, space="PSUM") as ps_wk, \
             tc.tile_pool(name="ps_out", bufs=2, space="PSUM") as ps_out:
            qT = persist.tile([128, 4 * S], BF)
            kT = persist.tile([128, 4 * S], BF)
            # v1: per (head h, k-chunk sck) a [128, 66] block at col
            # (h*16+sck)*66: cols 0:64 = v values, col 64 = ones (denom),
            # col 65 pad; +128 tail pad for the 128-col padded stationary.
            v1 = persist.tile([128, HG * 16 * VBLK + 128], BF)
            oT = obig_pool.tile([128, 4 * S], BF)
            nc.vector.memset(v1[:], 1.0)
            tri2 = persist.tile([128, 256], BF)
            nc.sync.dma_start(tri2[:], masks_d.ap())

            xT_tiles = {}

            def dma_x(tb):
                t = xT_pool.tile([128, KC * 512], BF, tag="xT",
                                 name=f"xTd_{tb}")
                for ki in range(KC):
                    nc.sync.dma_start(
                        t[:, ki * 512:(ki + 1) * 512],
                        xT_d[ki * 128:(ki + 1) * 128,
                             tb * 512:(tb + 1) * 512])
                xT_tiles[tb] = t

            # interleave x(0) and w-qk chunk DMAs so the first matmul chain
            # trickles in with the DMA stream; defer the w-v half + wout
            w_sb = persist.tile([128, KC * 1536], BF)
            xt0 = xT_pool.tile([128, KC * 512], BF, tag="xT", name="xTd_0")
            xT_tiles[0] = xt0
            for ki in range(KC):
                nc.sync.dma_start(
                    xt0[:, ki * 512:(ki + 1) * 512],
                    xT_d[ki * 128:(ki + 1) * 128, 0:512])
                nc.sync.dma_start(
                    w_sb[:, ki * 1536: ki * 1536 + 1024],
                    wqkv_d[ki * 128:(ki + 1) * 128, 0:1024])
            for ki in range(KC):
                nc.sync.dma_start(
                    w_sb[:, ki * 1536 + 1024:(ki + 1) * 1536],
                    wqkv_d[ki * 128:(ki + 1) * 128, 1024:1536])
            wout_sb = persist.tile([128, 4 * D], BF)

            def qkv_block(tb):
                xT_sb = xT_tiles.pop(tb)
                if tb + 1 < NB:
                    dma_x(tb + 1)
                # q (m 0..3) and k (m 4..7): stationary w chunk, moving xT
                for m in range(8):
                    acc = ps_wk.tile([128, 512], F32, tag="wk",
                                     name=f"acc_{tb}_{m}")
                    for ki in range(KC):
                        nc.tensor.matmul(
                            acc[:],
                            w_sb[:, ki * 1536 + m * 128: ki * 1536 + (m + 1) * 128],
                            xT_sb[:, ki * 512:(ki + 1) * 512],
                            start=(ki == 0), stop=(ki == KC - 1))
                    dst = qT if m < 4 else kT
                    r = m if m < 4 else m - 4
                    nc.vector.tensor_copy(
                        dst[:, r * S + tb * 512: r * S + tb * 512 + 512],
                        acc[:])
                # v natural: stationary xT token chunk, moving Wv
                for t in range(4):
                    vacc = ps_wk.tile([128, 512], F32, tag="wk",
                                      name=f"vacc_{tb}_{t}")
                    for ki in range(KC):
                        nc.tensor.matmul(
                            vacc[:],
                            xT_sb[:, ki * 512 + t * 128: ki * 512 + (t + 1) * 128],
                            w_sb[:, ki * 1536 + 1024: ki * 1536 + 1536],
                            start=(ki == 0), stop=(ki == KC - 1))
                    sck = tb * 4 + t
                    # strided copy: head h's 64 cols -> v1 block (h*16+sck)
                    v1v = v1[:, 0:HG * 16 * VBLK].rearrange(
                        "p (h c u) -> p h c u", h=HG, c=16)
                    nc.vector.tensor_copy(
                        v1v[:, :, sck:sck + 1, 0:64],
                        vacc[:].rearrange("p (h o u) -> p h o u", h=HG, o=1))

            def attention_block(tb, fillers=()):
                ns = 4 * tb + 4   # k chunks for this q block
                nd = 4 * tb       # non-diagonal chunk count (even)
                tri2v = tri2[:].rearrange("p (h u) -> p h u", h=2)
                out_ps = {}
                pr_refs = {}
                arenas = {}

                def emit_sc(r, s):
                    lo = max(128 * s - 512 * tb, 0)
                    n = 512 - lo
                    scp = ps_sc.tile([128, 1024], F32, tag="sc",
                                     name=f"sc_{tb}_{r}_{s}")
                    for half in range(2):
                        po = 64 * half
                        nc.tensor.matmul(
                            scp[:, half * 512: half * 512 + n],
                            kT[po:po + 64, r * S + s * 128: r * S + s * 128 + 128],
                            qT[po:po + 64,
                               r * S + 512 * tb + lo: r * S + 512 * (tb + 1)],
                            start=True, stop=True)
                    pr = pr_pool.tile([128, 1024], BF, tag="probs",
                                      bufs=5, name=f"pr_{tb}_{r}_{s}")
                    nc.scalar.activation(
                        pr[:].rearrange("p (h u) -> p h u", h=2)[:, :, 0:n],
                        scp[:].rearrange("p (h u) -> p h u", h=2)[:, :, 0:n],
                        Exp)
                    if s >= nd:  # diagonal chunk: causal mask multiply
                        prv = pr[:].rearrange("p (h u) -> p h u", h=2)
                        nc.vector.tensor_mul(
                            prv[:, :, 0:128], prv[:, :, 0:128], tri2v)
                    pr_refs[(r, s)] = (pr, 0)

                def emit_out(r, s):
                    lo = max(128 * s - 512 * tb, 0)
                    n = 512 - lo
                    prt, off = pr_refs.pop((r, s))
                    for half in range(2):
                        h = 2 * r + half
                        if s == 0:
                            out_ps[h] = ps_out.tile(
                                [128, 512], F32, tag="o", name=f"ops_{tb}_{h}")
                        blk = (h * 16 + s) * VBLK
                        nc.tensor.matmul(
                            out_ps[h][:, lo:512],
                            v1[:, blk: blk + 128],
                            prt[:, off + half * 512: off + half * 512 + n],
                            start=(s == 0), stop=(s == ns - 1))
                    if s == ns - 1:
                        for half in range(2):
                            h = 2 * r + half
                            po = 64 * half
                            op = out_ps.pop(h)
                            den = rc_pool.tile([1, 512], F32, tag="den")
                            nc.vector.tensor_copy(den[:], op[64:65, :])
                            rc = rc_pool.tile([1, 512], F32, tag="rc")
                            nc.vector.reciprocal_approx_fast(rc[:], den[:])
                            rb = rb_pool.tile([64, 512], F32, tag="rb")
                            nc.gpsimd.partition_broadcast(rb[:], rc[:])
                            nc.vector.tensor_mul(
                                oT[po:po + 64,
                                   r * S + 512 * tb: r * S + 512 * tb + 512],
                                op[0:64, :], rb[:])

                LA = 2
                slots = [(r, s) for r in range(4) for s in range(ns)]
                fillers = list(fillers)
                stride = max(1, len(slots) // len(fillers)) if fillers else 0
                fi = 0
                for i, (r, s) in enumerate(slots):
                    if fillers and fi < len(fillers) and i % stride == stride // 2:
                        fillers[fi]()
                        fi += 1
                    emit_sc(r, s)
                    if i >= LA:
                        emit_out(*slots[i - LA])
                for rs in slots[-LA:]:
                    emit_out(*rs)
                while fi < len(fillers):
                    fillers[fi]()
                    fi += 1

            def proj_chunk(m):
                ost = ost_pool.tile([128, 1024], F32, tag="ost",
                                    name=f"ost_{m}")
                for half in range(2):
                    pso = ps_wk.tile([128, 512], F32, tag="wk",
                                     name=f"pso_{m}_{half}")
                    for k in range(4):
                        nc.tensor.matmul(
                            pso[:],
                            oT[:, k * S + m * 128: k * S + m * 128 + 128],
                            wout_sb[:, k * D + half * 512: k * D + half * 512 + 512],
                            start=(k == 0), stop=(k == 3))
                    nc.vector.tensor_copy(
                        ost[:, half * 512:(half + 1) * 512], pso[:])
                nc.sync.dma_start(
                    out_d[m * 128:(m + 1) * 128, :], ost[:])

            def proj_fillers(j):
                return [lambda m=m: proj_chunk(m) for m in range(4 * j, 4 * j + 4)]

            for tb in range(NB):
                qkv_block(tb)
                if tb == 0:
                    nc.sync.dma_start(
                        wout_sb[:].rearrange("p (k n) -> p k n", k=4),
                        wout_d.ap().rearrange("(k p) n -> p k n", p=128),
                    )
                attention_block(
                    tb, proj_fillers(tb - 1) if tb >= 1 else ())
            for f in proj_fillers(NB - 1):
                f()
    nc.compile()
    return nc


def _make_masks():
    # tri2[p, j*128+c] = 1.0 if c >= p else 0 (keep-mask for the two
    # halves of a diagonal-chunk probs pair)
    p = np.arange(128)[:, None]
    c = np.arange(128)[None, :]
    tri = (c >= p).astype(np.float32)
    return np.concatenate([tri, tri], axis=1)


def _make_in_maps(x, w_qkv, w_out):
    import ml_dtypes
    bf = ml_dtypes.bfloat16
    masks = _make_masks().astype(bf)
    scale = np.float32(DH ** -0.5)
    in_maps = []
    for c in range(NCORES):
        g = c % 2
        wq = w_qkv[:, g * 512:(g + 1) * 512] * scale
        wk = w_qkv[:, D + g * 512: D + (g + 1) * 512]
        wv = w_qkv[:, 2 * D + g * 512: 2 * D + (g + 1) * 512]
        in_maps.append({
            "xT": np.ascontiguousarray(x[c // 2].T).astype(bf),
            "wqkv": np.ascontiguousarray(np.concatenate([wq, wk, wv], axis=1)).astype(bf),
            "wout": np.ascontiguousarray(w_out[g * 512:(g + 1) * 512, :]).astype(bf),
            "masks": masks,
        })
    return in_maps


def kernel(x, w_qkv, w_out):
    from concourse.bass_utils import run_bass_kernel_spmd

    x = np.asarray(x, dtype=np.float32)
    w_qkv = np.asarray(w_qkv, dtype=np.float32)
    w_out = np.asarray(w_out, dtype=np.float32)
    assert x.shape == (B, S, D) and w_qkv.shape == (D, 3 * D) and w_out.shape == (D, D)

    if "nc" not in _CACHE:
        _CACHE["nc"] = _build_nc()
    nc = _CACHE["nc"]

    in_maps = _make_in_maps(x, w_qkv, w_out)
    res = run_bass_kernel_spmd(nc, in_maps, core_ids=list(range(NCORES)),
                               trace=False)
    out = np.empty((B, S, D), dtype=np.float32)
    for b in range(B):
        out[b] = res.results[2 * b]["out"] + res.results[2 * b + 1]["out"]
    return out


# revision 19
# speedup vs baseline: 1.1628x; 1.0104x over previous
"""Causal self-attention on 8 trn2 NeuronCores.

Sharding (batch+head hint): core c handles batch b = c//2 (data parallel)
and head-group g = c%2 (8 of 16 heads; tensor-parallel slice of w_qkv
columns / w_out rows). Each core computes a full-batch-slice partial of the
output projection over its 512 head dims; the two partials per batch are
summed on gather (the "all-reduce after out_proj").

Kernel dataflow per core (S=2048 tokens, D=1024, 8 heads x 64):
  phase 1: x^T comes pre-transposed from the host. qT/kT = W^T @ x^T
           (stationary weights, moving tokens) in [64h, S] layouts; v is
           computed in NATURAL [tokens, vdim] orientation (stationary xT
           token chunks, moving Wv) and copied with a 66-col stride into
           v1 so each (head, chunk) block carries an appended ones column
           (denominator trick; v1 pre-memset to 1.0).
  phase 2: per head PAIR (even head on PE rows 0:64, odd head on rows
           64:128, adjacent matmuls -> concurrent row-group execution),
           exact-causal flash attention in transposed orientation:
           scoresT[k,q] pair -> one 2-bank PSUM tile; ONE ScalarE exp per
           pair (no max subtraction; scores O(N(0,1)) are fp32-safe);
           causal mask applied post-exp as a bf16 multiply on the diagonal
           chunks; out_hT[dh,q] += v1_blk.T @ probsT with the stationary
           padded to 128 cols (FWL-eligible weight loads), ones row at
           partition 64 accumulates the denominator; normalize via
           reciprocal + gpsimd partition_broadcast.
  phase 3: partial out = oT.T @ Wout (per 128-token chunk), PSUM->SBUF on
           ScalarE, paired 4KB-row DMA stores.
"""
import numpy as np

B = 4
S = 2048
D = 1024
HG = 8           # heads per core
DH = 64
NCORES = 8
NB = S // 512    # 512-token q blocks
KC = D // 128    # contraction chunks over D
VBLK = 72        # v1 block stride: 64 v cols + ones col + pad (16B-aligned)

_CACHE = {}


def _build_nc():
    import concourse.bass as bass  # noqa
    import concourse.mybir as mybir
    import concourse.tile as tile
    from concourse import bacc

    F32 = mybir.dt.float32
    BF = mybir.dt.bfloat16
    Exp = mybir.ActivationFunctionType.Exp

    nc = bacc.Bacc("TRN2", target_bir_lowering=False, debug=False,
                   enable_asserts=False, num_devices=NCORES)
    xT_d = nc.dram_tensor("xT", [D, S], BF, kind="ExternalInput")
    wqkv_d = nc.dram_tensor("wqkv", [D, 3 * 512], BF, kind="ExternalInput")
    wout_d = nc.dram_tensor("wout", [512, D], BF, kind="ExternalInput")
    masks_d = nc.dram_tensor("masks", [128, 256], BF, kind="ExternalInput")
    out_d = nc.dram_tensor("out", [S, D], F32, kind="ExternalOutput")

    with tile.TileContext(nc) as tc:
        with tc.tile_pool(name="persist", bufs=1) as persist, \
             tc.tile_pool(name="xT", bufs=2) as xT_pool, \
             tc.tile_pool(name="probs", bufs=6) as pr_pool, \
             tc.tile_pool(name="recip", bufs=2) as rc_pool, \
             tc.tile_pool(name="rbc", bufs=2) as rb_pool, \
             tc.tile_pool(name="obig", bufs=1) as obig_pool, \
             tc.tile_pool(name="ostage", bufs=3) as ost_pool, \
             tc.tile_pool(name="ps_sc", bufs=2, space="PSUM") as ps_sc, \
             tc.tile_pool(name="ps_wk", bufs=2, space="PSUM") as ps_wk, \
             tc.tile_pool(name="ps_out", bufs=2, space="PSUM") as ps_out:
            qT = persist.tile([128, 4 * S], BF)
            kT = persist.tile([128, 4 * S], BF)
            # v1: per (head h, k-chunk sck) a [128, 66] block at col
            # (h*16+sck)*66: cols 0:64 = v values, col 64 = ones (denom),
            # col 65 pad; +128 tail pad for the 128-col padded stationary.
            v1 = persist.tile([128, HG * 16 * VBLK + 128], BF)
            oT = obig_pool.tile([128, 4 * S], BF)
            nc.vector.memset(v1[:], 1.0)
            tri2 = persist.tile([128, 256], BF)
            nc.sync.dma_start(tri2[:], masks_d.ap())

            xT_tiles = {}

            def dma_x(tb):
                t = xT_pool.tile([128, KC * 512], BF, tag="xT",
                                 name=f"xTd_{tb}")
                for ki in range(KC):
                    nc.sync.dma_start(
                        t[:, ki * 512:(ki + 1) * 512],
                        xT_d[ki * 128:(ki + 1) * 128,
                             tb * 512:(tb + 1) * 512])
                xT_tiles[tb] = t

            # interleave x(0) and w-qk chunk DMAs so the first matmul chain
            # trickles in with the DMA stream; defer the w-v half + wout
            w_sb = persist.tile([128, KC * 1536], BF)
            xt0 = xT_pool.tile([128, KC * 512], BF, tag="xT", name="xTd_0")
            xT_tiles[0] = xt0
            for ki in range(KC):
                nc.sync.dma_start(
                    xt0[:, ki * 512:(ki + 1) * 512],
                    xT_d[ki * 128:(ki + 1) * 128, 0:512])
                nc.sync.dma_start(
                    w_sb[:, ki * 1536: ki * 1536 + 1024],
                    wqkv_d[ki * 128:(ki + 1) * 128, 0:1024])
            for ki in range(KC):
                nc.sync.dma_start(
                    w_sb[:, ki * 1536 + 1024:(ki + 1) * 1536],
                    wqkv_d[ki * 128:(ki + 1) * 128, 1024:1536])
            wout_sb = persist.tile([128, 4 * D], BF)

            def qkv_block(tb):
                xT_sb = xT_tiles.pop(tb)
                if tb + 1 < NB:
                    dma_x(tb + 1)
                # q (m 0..3) and k (m 4..7): stationary w chunk, moving xT
                for m in range(8):
                    acc = ps_wk.tile([128, 512], F32, tag="wk",
                                     name=f"acc_{tb}_{m}")
                    for ki in range(KC):
                        nc.tensor.matmul(
                            acc[:],
                            w_sb[:, ki * 1536 + m * 128: ki * 1536 + (m + 1) * 128],
                            xT_sb[:, ki * 512:(ki + 1) * 512],
                            start=(ki == 0), stop=(ki == KC - 1))
                    dst = qT if m < 4 else kT
                    r = m if m < 4 else m - 4
                    nc.vector.tensor_copy(
                        dst[:, r * S + tb * 512: r * S + tb * 512 + 512],
                        acc[:])
                # v natural: stationary xT token chunk, moving Wv
                for t in range(4):
                    vacc = ps_wk.tile([128, 512], F32, tag="wk",
                                      name=f"vacc_{tb}_{t}")
                    for ki in range(KC):
                        nc.tensor.matmul(
                            vacc[:],
                            xT_sb[:, ki * 512 + t * 128: ki * 512 + (t + 1) * 128],
                            w_sb[:, ki * 1536 + 1024: ki * 1536 + 1536],
                            start=(ki == 0), stop=(ki == KC - 1))
                    sck = tb * 4 + t
                    # strided copy: head h's 64 cols -> v1 block (h*16+sck)
                    v1v = v1[:, 0:HG * 16 * VBLK].rearrange(
                        "p (h c u) -> p h c u", h=HG, c=16)
                    nc.vector.tensor_copy(
                        v1v[:, :, sck:sck + 1, 0:64],
                        vacc[:].rearrange("p (h o u) -> p h o u", h=HG, o=1))

            def attention_block(tb, fillers=()):
                ns = 4 * tb + 4   # k chunks for this q block
                nd = 4 * tb       # non-diagonal chunk count (even)
                tri2v = tri2[:].rearrange("p (h u) -> p h u", h=2)
                out_ps = {}
                pr_refs = {}
                arenas = {}

                def emit_sc(r, s):
                    lo = max(128 * s - 512 * tb, 0)
                    n = 512 - lo
                    scp = ps_sc.tile([128, 1024], F32, tag="sc",
                                     name=f"sc_{tb}_{r}_{s}")
                    for half in range(2):
                        po = 64 * half
                        nc.tensor.matmul(
                            scp[:, half * 512: half * 512 + n],
                            kT[po:po + 64, r * S + s * 128: r * S + s * 128 + 128],
                            qT[po:po + 64,
                               r * S + 512 * tb + lo: r * S + 512 * (tb + 1)],
                            start=True, stop=True)
                    pr = pr_pool.tile([128, 1024], BF, tag="probs",
                                      bufs=5, name=f"pr_{tb}_{r}_{s}")
                    nc.scalar.activation(
                        pr[:].rearrange("p (h u) -> p h u", h=2)[:, :, 0:n],
                        scp[:].rearrange("p (h u) -> p h u", h=2)[:, :, 0:n],
                        Exp)
                    if s >= nd:  # diagonal chunk: causal mask multiply
                        prv = pr[:].rearrange("p (h u) -> p h u", h=2)
                        nc.vector.tensor_mul(
                            prv[:, :, 0:128], prv[:, :, 0:128], tri2v)
                    pr_refs[(r, s)] = (pr, 0)

                def emit_out(r, s):
                    lo = max(128 * s - 512 * tb, 0)
                    n = 512 - lo
                    prt, off = pr_refs.pop((r, s))
                    for half in range(2):
                        h = 2 * r + half
                        if s == 0:
                            out_ps[h] = ps_out.tile(
                                [128, 512], F32, tag="o", name=f"ops_{tb}_{h}")
                        blk = (h * 16 + s) * VBLK
                        nc.tensor.matmul(
                            out_ps[h][:, lo:512],
                            v1[:, blk: blk + 128],
                            prt[:, off + half * 512: off + half * 512 + n],
                            start=(s == 0), stop=(s == ns - 1))
                    if s == ns - 1:
                        for half in range(2):
                            h = 2 * r + half
                            po = 64 * half
                            op = out_ps.pop(h)
                            den = rc_pool.tile([1, 512], F32, tag="den")
                            nc.vector.tensor_copy(den[:], op[64:65, :])
                            rc = rc_pool.tile([1, 512], F32, tag="rc")
                            nc.vector.reciprocal_approx_fast(rc[:], den[:])
                            rb = rb_pool.tile([64, 512], F32, tag="rb")
                            nc.gpsimd.partition_broadcast(rb[:], rc[:])
                            nc.vector.tensor_mul(
                                oT[po:po + 64,
                                   r * S + 512 * tb: r * S + 512 * tb + 512],
                                op[0:64, :], rb[:])

                LA = 3
                slots = [(r, s) for r in range(4) for s in range(ns)]
                fillers = list(fillers)
                stride = max(1, len(slots) // len(fillers)) if fillers else 0
                fi = 0
                for i, (r, s) in enumerate(slots):
                    if fillers and fi < len(fillers) and i % stride == stride // 2:
                        fillers[fi]()
                        fi += 1
                    emit_sc(r, s)
                    if i >= LA:
                        emit_out(*slots[i - LA])
                for rs in slots[-LA:]:
                    emit_out(*rs)
                while fi < len(fillers):
                    fillers[fi]()
                    fi += 1

            def proj_chunk(m):
                ost = ost_pool.tile([128, 1024], F32, tag="ost",
                                    name=f"ost_{m}")
                for half in range(2):
                    pso = ps_wk.tile([128, 512], F32, tag="wk",
                                     name=f"pso_{m}_{half}")
                    for k in range(4):
                        nc.tensor.matmul(
                            pso[:],
                            oT[:, k * S + m * 128: k * S + m * 128 + 128],
                            wout_sb[:, k * D + half * 512: k * D + half * 512 + 512],
                            start=(k == 0), stop=(k == 3))
                    nc.vector.tensor_copy(
                        ost[:, half * 512:(half + 1) * 512], pso[:])
                nc.sync.dma_start(
                    out_d[m * 128:(m + 1) * 128, :], ost[:])

            def proj_fillers(j):
                return [lambda m=m: proj_chunk(m) for m in range(4 * j, 4 * j + 4)]

            for tb in range(NB):
                qkv_block(tb)
                if tb == 0:
                    nc.sync.dma_start(
                        wout_sb[:].rearrange("p (k n) -> p k n", k=4),
                        wout_d.ap().rearrange("(k p) n -> p k n", p=128),
                    )
                attention_block(
                    tb, proj_fillers(tb - 1) if tb >= 1 else ())
            for f in proj_fillers(NB - 1):
                f()
    nc.compile()
    return nc


def _make_masks():
    # tri2[p, j*128+c] = 1.0 if c >= p else 0 (keep-mask for the two
    # halves of a diagonal-chunk probs pair)
    p = np.arange(128)[:, None]
    c = np.arange(128)[None, :]
    tri = (c >= p).astype(np.float32)
    return np.concatenate([tri, tri], axis=1)


def _make_in_maps(x, w_qkv, w_out):
    import ml_dtypes
    bf = ml_dtypes.bfloat16
    masks = _make_masks().astype(bf)
    scale = np.float32(DH ** -0.5)
    in_maps = []
    for c in range(NCORES):
        g = c % 2
        wq = w_qkv[:, g * 512:(g + 1) * 512] * scale
        wk = w_qkv[:, D + g * 512: D + (g + 1) * 512]
        wv = w_qkv[:, 2 * D + g * 512: 2 * D + (g + 1) * 512]
        in_maps.append({
            "xT": np.ascontiguousarray(x[c // 2].T).astype(bf),
            "wqkv": np.ascontiguousarray(np.concatenate([wq, wk, wv], axis=1)).astype(bf),
            "wout": np.ascontiguousarray(w_out[g * 512:(g + 1) * 512, :]).astype(bf),
            "masks": masks,
        })
    return in_maps


def kernel(x, w_qkv, w_out):
    from concourse.bass_utils import run_bass_kernel_spmd

    x = np.asarray(x, dtype=np.float32)
    w_qkv = np.asarray(w_qkv, dtype=np.float32)
    w_out = np.asarray(w_out, dtype=np.float32)
    assert x.shape == (B, S, D) and w_qkv.shape == (D, 3 * D) and w_out.shape == (D, D)

    if "nc" not in _CACHE:
        _CACHE["nc"] = _build_nc()
    nc = _CACHE["nc"]

    in_maps = _make_in_maps(x, w_qkv, w_out)
    res = run_bass_kernel_spmd(nc, in_maps, core_ids=list(range(NCORES)),
                               trace=False)
    out = np.empty((B, S, D), dtype=np.float32)
    for b in range(B):
        out[b] = res.results[2 * b]["out"] + res.results[2 * b + 1]["out"]
    return out
